# revision 2
# baseline (speedup 1.0000x reference)
"""MaxPool3d (kernel=3, stride=2, padding=1) on Trainium2, 8 NeuronCores. v4.

Input  x: (2, 32, 128, 128, 128) f32  ->  Output: (2, 32, 64, 64, 64) f32.

Sharding: 64 (b, c) slices data-parallel; each core gets 8 slices as 4
slice-pairs (a pair packs 2 slices into the 128 SBUF partitions: partition
64*s + d/2, parity slabs for even/odd d).

Pipeline (per h-chunk, O parity first so the partition-shift DMA hides
under the E-parity compute):
  - SWDGE cast-load f32->bf16 (HBM line rate, halves SBUF traffic).
  - H pool on DVE in 2x mode (bf16, unit-stride rows).
  - W pool on DVE (strided, 1x) into per-chunk Ge/Go tiles.
  - partition-shift of Go via SBUF->SBUF DMA on the sync queue (Gs).
  - D pool on DVE: Ge = max(Ge, Go) in 2x; Yt_f32 = max(Ge, Gs) (f32 out,
    1x) -- the final op also does the bf16->f32 cast for free.
  - store Yt from the scalar queue (ACT is a pure DMA-trigger engine here).

No tile is shared across chunks (no accumulator), so chunks only couple
through pool-buffer rotation; bufs>=2 keeps every stage double-buffered,
and xpool bufs=3 rides through the chunk-boundary row carry (xprev).

Window math (PADDING=1): out[o] = max(in[2o-1], in[2o], in[2o+1]).
D axis: out[od] = max(E[od], O[od], O[od-1]); O[od-1] = Gs partition shift;
partitions 0/64 of Gs duplicate Go rows 0/64 (idempotent under max).
"""

import sys

sys.path.insert(0, "/opt/trn_rl_repo")

import numpy as np

B, C, D, H, W = 2, 32, 128, 128, 128
OD, OH, OW = 64, 64, 64
N_CORES = 8
SLICES_PER_CORE = (B * C) // N_CORES  # 8
PAIRS = SLICES_PER_CORE // 2  # 4
HCMAX = 64
# per-pair chunk schedules: small first chunks start DVE early (ramp);
# small last chunks shrink the after-last-load tail
CHUNKS = [
    [32, 32, 64],  # pair 0: ramp
    [64, 64],
    [64, 64],
    [64, 48, 16],  # pair 3: tail
]
assert all(sum(cs) == H and max(cs) <= HCMAX for cs in CHUNKS)

_cache = {}
INST_LABELS = {}


def _lab(inst, label):
    INST_LABELS[inst.ins.name] = label
    return inst


def _build():
    import concourse.mybir as mybir
    from concourse import bacc
    from concourse.tile import TileContext

    f32 = mybir.dt.float32
    bf16 = mybir.dt.bfloat16
    nc = bacc.Bacc()
    x_ext = nc.declare_dram_parameter(
        "x_shard", [SLICES_PER_CORE, D, H, W], f32, isOutput=False
    )
    y_ext = nc.declare_dram_parameter(
        "y_shard", [SLICES_PER_CORE, OD, OH, OW], f32, isOutput=True
    )

    OHCMAX = HCMAX // 2

    with TileContext(nc) as tc:
        with (
            tc.tile_pool(name="xpool", bufs=3) as xpool,
            tc.tile_pool(name="hpool", bufs=2) as hpool,
            tc.tile_pool(name="gpool", bufs=3) as gpool,
            tc.tile_pool(name="spool", bufs=3) as spool,
            tc.tile_pool(name="ypool", bufs=3) as ypool,
            tc.tile_pool(name="cpool", bufs=2) as cpool,
        ):
            for p in range(PAIRS):
                s0 = 2 * p
                xprev = {0: None, 1: None}
                h0 = 0
                for c, hc in enumerate(CHUNKS[p]):
                    ohc = hc // 2
                    oh0 = h0 // 2
                    ohr = slice(oh0, oh0 + ohc)
                    G = {}
                    for par in (1, 0):  # O first: shift DMA overlaps E compute
                        nm = "E" if par == 0 else "O"
                        xt = xpool.tile(
                            [128, HCMAX, W], bf16, name=f"x{nm}", tag=f"x{nm}"
                        )
                        # cast load f32 -> bf16 (SWDGE)
                        _lab(nc.gpsimd.dma_start(
                            out=xt[:, 0:hc, :],
                            in_=x_ext[s0 : s0 + 2, par : D : 2, h0 : h0 + hc, :],
                        ), f"p{p}c{c}ld{nm}")
                        # ---- H pool (free axis, unit-stride rows, 2x) ----
                        Ht = hpool.tile(
                            [128, OHCMAX, W], bf16, name=f"H{nm}", tag=f"H{nm}"
                        )
                        _lab(nc.vector.tensor_max(
                            out=Ht[:, 0:ohc, :],
                            in0=xt[:, 0:hc:2, :],
                            in1=xt[:, 1:hc:2, :],
                        ), f"p{p}c{c}H1{nm}")
                        _lab(nc.vector.tensor_max(
                            out=Ht[:, 1:ohc, :],
                            in0=Ht[:, 1:ohc, :],
                            in1=xt[:, 1 : hc - 2 : 2, :],
                        ), f"p{p}c{c}H2{nm}")
                        if c > 0:
                            # boundary: h = 2*oh0 - 1 = prev chunk's last row
                            _lab(nc.vector.tensor_max(
                                out=Ht[:, 0:1, :],
                                in0=Ht[:, 0:1, :],
                                in1=xprev[par][:, 0:1, :],
                            ), f"p{p}c{c}bnd{nm}")
                        # copy the carry row out so xt's lifetime ends inside
                        # this chunk (otherwise the next chunk's boundary op
                        # pins the buffer and stalls the load stream)
                        if c < len(CHUNKS[p]) - 1:
                            cw = cpool.tile(
                                [128, 1, W], bf16, name=f"c{nm}", tag=f"c{nm}"
                            )
                            _lab(nc.vector.tensor_copy(
                                out=cw[:, 0:1, :], in_=xt[:, hc - 1 : hc, :]
                            ), f"p{p}c{c}cw{nm}")
                            xprev[par] = cw
                        # ---- W pool (strided, 1x) ----
                        Gt = gpool.tile(
                            [128, OHCMAX, OW], bf16, name=f"G{nm}", tag=f"G{nm}"
                        )
                        _lab(nc.vector.tensor_max(
                            out=Gt[:, 0:ohc, :],
                            in0=Ht[:, 0:ohc, 0:W:2],
                            in1=Ht[:, 0:ohc, 1:W:2],
                        ), f"p{p}c{c}W1{nm}")
                        _lab(nc.vector.tensor_max(
                            out=Gt[:, 0:ohc, 1:OW],
                            in0=Gt[:, 0:ohc, 1:OW],
                            in1=Ht[:, 0:ohc, 1 : W - 2 : 2],
                        ), f"p{p}c{c}W2{nm}")
                        G[par] = Gt
                        if par == 1:
                            # partition-shift of Go on the sync queue; runs
                            # while DVE does the E-parity H/W ops
                            Gs = spool.tile(
                                [128, OHCMAX, OW], bf16, name="Gs", tag="Gs"
                            )
                            _lab(nc.sync.dma_start(
                                out=Gs[1:64, 0:ohc, :], in_=Gt[0:63, 0:ohc, :]
                            ), f"p{p}c{c}sh1")
                            _lab(nc.sync.dma_start(
                                out=Gs[65:128, 0:ohc, :],
                                in_=Gt[64:127, 0:ohc, :],
                            ), f"p{p}c{c}sh2")
                            _lab(nc.sync.dma_start(
                                out=Gs[0:65:64, 0:ohc, :],
                                in_=Gt[0:65:64, 0:ohc, :],
                            ), f"p{p}c{c}sh3")
                    # ---- D pool (partition axis), all bf16 2x in-place ----
                    Ge, Go = G[0], G[1]
                    _lab(nc.vector.tensor_max(
                        out=Ge[:, 0:ohc, :],
                        in0=Ge[:, 0:ohc, :],
                        in1=Go[:, 0:ohc, :],
                    ), f"p{p}c{c}D1")
                    _lab(nc.vector.tensor_max(
                        out=Ge[:, 0:ohc, :],
                        in0=Ge[:, 0:ohc, :],
                        in1=Gs[:, 0:ohc, :],
                    ), f"p{p}c{c}D2")
                    # ---- cast bf16 -> f32 on the idle ACT engine, then
                    # store (trigger follows the cast on the same engine) ----
                    Yt = ypool.tile([128, OHCMAX, OW], f32, name="Yt", tag="Yt")
                    _lab(nc.scalar.activation(
                        out=Yt[:, 0:ohc, :],
                        in_=Ge[:, 0:ohc, :],
                        func=mybir.ActivationFunctionType.Copy,
                    ), f"p{p}c{c}cast")
                    _lab(nc.scalar.dma_start(
                        out=y_ext[s0 : s0 + 2, :, ohr, :], in_=Yt[:, 0:ohc, :]
                    ), f"p{p}c{c}st")
                    h0 += hc
    nc.compile()
    return nc


def _get_nc():
    if "nc" not in _cache:
        _cache["nc"] = _build()
    return _cache["nc"]


def run(x: np.ndarray, **spmd_kwargs):
    """Run the SPMD kernel; returns the BassKernelResults (for tracing)."""
    from concourse.bass_utils import run_bass_kernel_spmd

    nc = _get_nc()
    xs = np.ascontiguousarray(x, dtype=np.float32).reshape(B * C, D, H, W)
    in_maps = [
        {"x_shard": np.ascontiguousarray(xs[SLICES_PER_CORE * i : SLICES_PER_CORE * (i + 1)])}
        for i in range(N_CORES)
    ]
    return run_bass_kernel_spmd(nc, in_maps, list(range(N_CORES)), **spmd_kwargs)


def kernel(x: np.ndarray) -> np.ndarray:
    res = run(x)
    out = np.stack([res.results[i]["y_shard"] for i in range(N_CORES)])
    return out.reshape(B, C, OD, OH, OW)


# revision 3
# speedup vs baseline: 1.2204x; 1.2204x over previous
"""MaxPool3d (kernel=3, stride=2, padding=1) on Trainium2, 8 NeuronCores. v4.

Input  x: (2, 32, 128, 128, 128) f32  ->  Output: (2, 32, 64, 64, 64) f32.

Sharding: 64 (b, c) slices data-parallel; each core gets 8 slices as 4
slice-pairs (a pair packs 2 slices into the 128 SBUF partitions: partition
64*s + d/2, parity slabs for even/odd d).

Pipeline (per h-chunk, O parity first so the partition-shift DMA hides
under the E-parity compute):
  - SWDGE cast-load f32->bf16 (HBM line rate, halves SBUF traffic).
  - H pool on DVE in 2x mode (bf16, unit-stride rows).
  - W pool on DVE (strided, 1x) into per-chunk Ge/Go tiles.
  - partition-shift of Go via SBUF->SBUF DMA on the sync queue (Gs).
  - D pool on DVE: Ge = max(Ge, Go) in 2x; Yt_f32 = max(Ge, Gs) (f32 out,
    1x) -- the final op also does the bf16->f32 cast for free.
  - store Yt from the scalar queue (ACT is a pure DMA-trigger engine here).

No tile is shared across chunks (no accumulator), so chunks only couple
through pool-buffer rotation; bufs>=2 keeps every stage double-buffered,
and xpool bufs=3 rides through the chunk-boundary row carry (xprev).

Window math (PADDING=1): out[o] = max(in[2o-1], in[2o], in[2o+1]).
D axis: out[od] = max(E[od], O[od], O[od-1]); O[od-1] = Gs partition shift;
partitions 0/64 of Gs duplicate Go rows 0/64 (idempotent under max).
"""

import sys

sys.path.insert(0, "/opt/trn_rl_repo")

import numpy as np

B, C, D, H, W = 2, 32, 128, 128, 128
OD, OH, OW = 64, 64, 64
N_CORES = 8
SLICES_PER_CORE = (B * C) // N_CORES  # 8
PAIRS = SLICES_PER_CORE // 2  # 4
HCMAX = 64
# per-pair chunk schedules: small first chunks start DVE early (ramp);
# small last chunks shrink the after-last-load tail
CHUNKS = [
    [32, 32, 64],  # pair 0: ramp
    [64, 64],
    [64, 64],
    [64, 48, 16],  # pair 3: tail
]
assert all(sum(cs) == H and max(cs) <= HCMAX for cs in CHUNKS)

_cache = {}
INST_LABELS = {}


def _lab(inst, label):
    INST_LABELS[inst.ins.name] = label
    return inst


def _build():
    import concourse.mybir as mybir
    from concourse import bacc
    from concourse.tile import TileContext

    f32 = mybir.dt.float32
    bf16 = mybir.dt.bfloat16
    nc = bacc.Bacc()
    x_ext = nc.declare_dram_parameter(
        "x_shard", [SLICES_PER_CORE, D, H, W], f32, isOutput=False
    )
    y_ext = nc.declare_dram_parameter(
        "y_shard", [SLICES_PER_CORE, OD, OH, OW], f32, isOutput=True
    )

    OHCMAX = HCMAX // 2

    with TileContext(nc) as tc:
        with (
            tc.tile_pool(name="xpool", bufs=3) as xpool,
            tc.tile_pool(name="hpool", bufs=2) as hpool,
            tc.tile_pool(name="gpool", bufs=3) as gpool,
            tc.tile_pool(name="spool", bufs=3) as spool,
            tc.tile_pool(name="ypool", bufs=3) as ypool,
            tc.tile_pool(name="cpool", bufs=2) as cpool,
        ):
            pend = None

            def _flush_tail(t):
                fp, fc, Ge, Go, Gs, fohr, fohc, fs0 = t
                _lab(nc.vector.tensor_max(
                    out=Ge[:, 0:fohc, :],
                    in0=Ge[:, 0:fohc, :],
                    in1=Go[:, 0:fohc, :],
                ), f"p{fp}c{fc}D1")
                _lab(nc.vector.tensor_max(
                    out=Ge[:, 0:fohc, :],
                    in0=Ge[:, 0:fohc, :],
                    in1=Gs[:, 0:fohc, :],
                ), f"p{fp}c{fc}D2")
                Yt = ypool.tile([128, OHCMAX, OW], f32, name="Yt", tag="Yt")
                _lab(nc.scalar.activation(
                    out=Yt[:, 0:fohc, :],
                    in_=Ge[:, 0:fohc, :],
                    func=mybir.ActivationFunctionType.Copy,
                ), f"p{fp}c{fc}cast")
                _lab(nc.scalar.dma_start(
                    out=y_ext[fs0 : fs0 + 2, :, fohr, :], in_=Yt[:, 0:fohc, :]
                ), f"p{fp}c{fc}st")

            for p in range(PAIRS):
                s0 = 2 * p
                xprev = {0: None, 1: None}
                h0 = 0
                for c, hc in enumerate(CHUNKS[p]):
                    ohc = hc // 2
                    oh0 = h0 // 2
                    ohr = slice(oh0, oh0 + ohc)
                    G = {}
                    for par in (1, 0):  # O first: shift DMA overlaps E compute
                        nm = "E" if par == 0 else "O"
                        xt = xpool.tile(
                            [128, HCMAX, W], bf16, name=f"x{nm}", tag=f"x{nm}"
                        )
                        # cast load f32 -> bf16 (SWDGE)
                        _lab(nc.gpsimd.dma_start(
                            out=xt[:, 0:hc, :],
                            in_=x_ext[s0 : s0 + 2, par : D : 2, h0 : h0 + hc, :],
                        ), f"p{p}c{c}ld{nm}")
                        # ---- H pool (free axis, unit-stride rows, 2x) ----
                        Ht = hpool.tile(
                            [128, OHCMAX, W], bf16, name=f"H{nm}", tag=f"H{nm}"
                        )
                        _lab(nc.vector.tensor_max(
                            out=Ht[:, 0:ohc, :],
                            in0=xt[:, 0:hc:2, :],
                            in1=xt[:, 1:hc:2, :],
                        ), f"p{p}c{c}H1{nm}")
                        _lab(nc.vector.tensor_max(
                            out=Ht[:, 1:ohc, :],
                            in0=Ht[:, 1:ohc, :],
                            in1=xt[:, 1 : hc - 2 : 2, :],
                        ), f"p{p}c{c}H2{nm}")
                        if c > 0:
                            # boundary: h = 2*oh0 - 1 = prev chunk's last row
                            _lab(nc.vector.tensor_max(
                                out=Ht[:, 0:1, :],
                                in0=Ht[:, 0:1, :],
                                in1=xprev[par][:, 0:1, :],
                            ), f"p{p}c{c}bnd{nm}")
                        # copy the carry row out so xt's lifetime ends inside
                        # this chunk (otherwise the next chunk's boundary op
                        # pins the buffer and stalls the load stream)
                        if c < len(CHUNKS[p]) - 1:
                            cw = cpool.tile(
                                [128, 1, W], bf16, name=f"c{nm}", tag=f"c{nm}"
                            )
                            _lab(nc.vector.tensor_copy(
                                out=cw[:, 0:1, :], in_=xt[:, hc - 1 : hc, :]
                            ), f"p{p}c{c}cw{nm}")
                            xprev[par] = cw
                        # ---- W pool (strided, 1x) ----
                        Gt = gpool.tile(
                            [128, OHCMAX, OW], bf16, name=f"G{nm}", tag=f"G{nm}"
                        )
                        _lab(nc.vector.tensor_max(
                            out=Gt[:, 0:ohc, :],
                            in0=Ht[:, 0:ohc, 0:W:2],
                            in1=Ht[:, 0:ohc, 1:W:2],
                        ), f"p{p}c{c}W1{nm}")
                        _lab(nc.vector.tensor_max(
                            out=Gt[:, 0:ohc, 1:OW],
                            in0=Gt[:, 0:ohc, 1:OW],
                            in1=Ht[:, 0:ohc, 1 : W - 2 : 2],
                        ), f"p{p}c{c}W2{nm}")
                        G[par] = Gt
                        if par == 1:
                            # partition-shift of Go on the sync queue; runs
                            # while DVE does the E-parity H/W ops
                            Gs = spool.tile(
                                [128, OHCMAX, OW], bf16, name="Gs", tag="Gs"
                            )
                            _lab(nc.sync.dma_start(
                                out=Gs[1:64, 0:ohc, :], in_=Gt[0:63, 0:ohc, :]
                            ), f"p{p}c{c}sh1")
                            _lab(nc.sync.dma_start(
                                out=Gs[65:128, 0:ohc, :],
                                in_=Gt[64:127, 0:ohc, :],
                            ), f"p{p}c{c}sh2")
                            _lab(nc.sync.dma_start(
                                out=Gs[0:65:64, 0:ohc, :],
                                in_=Gt[0:65:64, 0:ohc, :],
                            ), f"p{p}c{c}sh3")
                    # ---- defer the D pool + cast + store by one chunk:
                    # by the time DVE reaches them, their DMA-completion
                    # semaphores (shift) are long satisfied, so the
                    # conservative vector-clock thresholds never stall ----
                    if pend is not None:
                        _flush_tail(pend)
                    pend = (p, c, G[0], G[1], Gs, ohr, ohc, s0)
                    h0 += hc
            if pend is not None:
                _flush_tail(pend)
    nc.compile()
    return nc


def _get_nc():
    if "nc" not in _cache:
        _cache["nc"] = _build()
    return _cache["nc"]


def run(x: np.ndarray, **spmd_kwargs):
    """Run the SPMD kernel; returns the BassKernelResults (for tracing)."""
    from concourse.bass_utils import run_bass_kernel_spmd

    nc = _get_nc()
    xs = np.ascontiguousarray(x, dtype=np.float32).reshape(B * C, D, H, W)
    in_maps = [
        {"x_shard": np.ascontiguousarray(xs[SLICES_PER_CORE * i : SLICES_PER_CORE * (i + 1)])}
        for i in range(N_CORES)
    ]
    return run_bass_kernel_spmd(nc, in_maps, list(range(N_CORES)), **spmd_kwargs)


def kernel(x: np.ndarray) -> np.ndarray:
    res = run(x)
    out = np.stack([res.results[i]["y_shard"] for i in range(N_CORES)])
    return out.reshape(B, C, OD, OH, OW)


# revision 5
# speedup vs baseline: 1.2217x; 1.0010x over previous
"""MaxPool3d (kernel=3, stride=2, padding=1) on Trainium2, 8 NeuronCores.

Input  x: (2, 32, 128, 128, 128) f32  ->  Output: (2, 32, 64, 64, 64) f32.

Sharding: 64 (b, c) slices data-parallel; each core gets 8 slices as 4
slice-pairs (a pair packs 2 slices into the 128 SBUF partitions: partition
64*s + d/2, parity slabs for even/odd d).

Pipeline (per h-chunk, O parity first so the partition-shift DMA hides
under the E-parity compute):
  - SWDGE cast-load f32->bf16 (HBM line rate, halves SBUF traffic).
  - H pool on DVE in 2x mode (bf16, unit-stride rows); the chunk-boundary
    row is copied to a tiny carry tile on ACT so the big x tiles retire
    inside their own chunk and never stall the load stream.
  - W pool on DVE (strided, 1x) into per-chunk Ge/Go tiles.
  - partition-shift of Go via SBUF->SBUF DMA on the sync queue (Gs).
  - D pool (Ge = max(Ge, Go, Gs), bf16 2x in-place), the bf16->f32 cast on
    the idle ACT engine, and the store (scalar queue) are DEFERRED two
    chunks behind the H/W front: their DMA-completion semaphore thresholds
    (conservative vector-clock joins that can fold in a later load's tick)
    are then long satisfied when DVE reaches them, so they never stall.

No tile is shared across chunks (no accumulator); stages couple only
through pool-buffer rotation, with enough bufs for the 2-chunk lag.

Window math (PADDING=1): out[o] = max(in[2o-1], in[2o], in[2o+1]).
D axis: out[od] = max(E[od], O[od], O[od-1]); O[od-1] = Gs partition shift;
partitions 0/64 of Gs duplicate Go rows 0/64 (idempotent under max).
"""

import sys

sys.path.insert(0, "/opt/trn_rl_repo")

import numpy as np

B, C, D, H, W = 2, 32, 128, 128, 128
OD, OH, OW = 64, 64, 64
N_CORES = 8
SLICES_PER_CORE = (B * C) // N_CORES  # 8
PAIRS = SLICES_PER_CORE // 2  # 4
HCMAX = 64
# per-pair chunk schedules: small first chunks start DVE early (ramp);
# small last chunks shrink the after-last-load tail
CHUNKS = [
    [32, 32, 64],  # pair 0: ramp
    [64, 64],
    [64, 64],
    [64, 48, 16],  # pair 3: tail
]
assert all(sum(cs) == H and max(cs) <= HCMAX for cs in CHUNKS)

_cache = {}
INST_LABELS = {}


def _lab(inst, label):
    INST_LABELS[inst.ins.name] = label
    return inst


def _build():
    import concourse.mybir as mybir
    from concourse import bacc
    from concourse.tile import TileContext

    f32 = mybir.dt.float32
    bf16 = mybir.dt.bfloat16
    nc = bacc.Bacc()
    x_ext = nc.declare_dram_parameter(
        "x_shard", [SLICES_PER_CORE, D, H, W], f32, isOutput=False
    )
    y_ext = nc.declare_dram_parameter(
        "y_shard", [SLICES_PER_CORE, OD, OH, OW], f32, isOutput=True
    )

    OHCMAX = HCMAX // 2

    with TileContext(nc) as tc:
        with (
            tc.tile_pool(name="xpool", bufs=3) as xpool,
            tc.tile_pool(name="hpool", bufs=2) as hpool,
            tc.tile_pool(name="gpool", bufs=4) as gpool,
            tc.tile_pool(name="spool", bufs=4) as spool,
            tc.tile_pool(name="ypool", bufs=3) as ypool,
            tc.tile_pool(name="cpool", bufs=2) as cpool,
        ):
            pend = []

            def _flush_tail(t):
                fp, fc, Ge, Go, Gs, fohr, fohc, fs0 = t
                _lab(nc.vector.tensor_max(
                    out=Ge[:, 0:fohc, :],
                    in0=Ge[:, 0:fohc, :],
                    in1=Go[:, 0:fohc, :],
                ), f"p{fp}c{fc}D1")
                _lab(nc.vector.tensor_max(
                    out=Ge[:, 0:fohc, :],
                    in0=Ge[:, 0:fohc, :],
                    in1=Gs[:, 0:fohc, :],
                ), f"p{fp}c{fc}D2")
                Yt = ypool.tile([128, OHCMAX, OW], f32, name="Yt", tag="Yt")
                _lab(nc.scalar.activation(
                    out=Yt[:, 0:fohc, :],
                    in_=Ge[:, 0:fohc, :],
                    func=mybir.ActivationFunctionType.Copy,
                ), f"p{fp}c{fc}cast")
                _lab(nc.scalar.dma_start(
                    out=y_ext[fs0 : fs0 + 2, :, fohr, :], in_=Yt[:, 0:fohc, :]
                ), f"p{fp}c{fc}st")

            for p in range(PAIRS):
                s0 = 2 * p
                xprev = {0: None, 1: None}
                h0 = 0
                for c, hc in enumerate(CHUNKS[p]):
                    ohc = hc // 2
                    oh0 = h0 // 2
                    ohr = slice(oh0, oh0 + ohc)
                    G = {}
                    for par in (1, 0):  # O first: shift DMA overlaps E compute
                        nm = "E" if par == 0 else "O"
                        xt = xpool.tile(
                            [128, HCMAX, W], bf16, name=f"x{nm}", tag=f"x{nm}"
                        )
                        # cast load f32 -> bf16 (SWDGE)
                        _lab(nc.gpsimd.dma_start(
                            out=xt[:, 0:hc, :],
                            in_=x_ext[s0 : s0 + 2, par : D : 2, h0 : h0 + hc, :],
                        ), f"p{p}c{c}ld{nm}")
                        # ---- H pool (free axis, unit-stride rows, 2x) ----
                        Ht = hpool.tile(
                            [128, OHCMAX, W], bf16, name=f"H{nm}", tag=f"H{nm}"
                        )
                        _lab(nc.vector.tensor_max(
                            out=Ht[:, 0:ohc, :],
                            in0=xt[:, 0:hc:2, :],
                            in1=xt[:, 1:hc:2, :],
                        ), f"p{p}c{c}H1{nm}")
                        _lab(nc.vector.tensor_max(
                            out=Ht[:, 1:ohc, :],
                            in0=Ht[:, 1:ohc, :],
                            in1=xt[:, 1 : hc - 2 : 2, :],
                        ), f"p{p}c{c}H2{nm}")
                        if c > 0:
                            # boundary: h = 2*oh0 - 1 = prev chunk's last row
                            _lab(nc.vector.tensor_max(
                                out=Ht[:, 0:1, :],
                                in0=Ht[:, 0:1, :],
                                in1=xprev[par][:, 0:1, :],
                            ), f"p{p}c{c}bnd{nm}")
                        # copy the carry row out so xt's lifetime ends inside
                        # this chunk (otherwise the next chunk's boundary op
                        # pins the buffer and stalls the load stream)
                        if c < len(CHUNKS[p]) - 1:
                            cw = cpool.tile(
                                [128, 1, W], bf16, name=f"c{nm}", tag=f"c{nm}"
                            )
                            _lab(nc.scalar.activation(
                                out=cw[:, 0:1, :],
                                in_=xt[:, hc - 1 : hc, :],
                                func=mybir.ActivationFunctionType.Copy,
                            ), f"p{p}c{c}cw{nm}")
                            xprev[par] = cw
                        # ---- W pool (strided, 1x) ----
                        Gt = gpool.tile(
                            [128, OHCMAX, OW], bf16, name=f"G{nm}", tag=f"G{nm}"
                        )
                        _lab(nc.vector.tensor_max(
                            out=Gt[:, 0:ohc, :],
                            in0=Ht[:, 0:ohc, 0:W:2],
                            in1=Ht[:, 0:ohc, 1:W:2],
                        ), f"p{p}c{c}W1{nm}")
                        _lab(nc.vector.tensor_max(
                            out=Gt[:, 0:ohc, 1:OW],
                            in0=Gt[:, 0:ohc, 1:OW],
                            in1=Ht[:, 0:ohc, 1 : W - 2 : 2],
                        ), f"p{p}c{c}W2{nm}")
                        G[par] = Gt
                        if par == 1:
                            # partition-shift of Go on the sync queue; runs
                            # while DVE does the E-parity H/W ops
                            Gs = spool.tile(
                                [128, OHCMAX, OW], bf16, name="Gs", tag="Gs"
                            )
                            _lab(nc.sync.dma_start(
                                out=Gs[1:64, 0:ohc, :], in_=Gt[0:63, 0:ohc, :]
                            ), f"p{p}c{c}sh1")
                            _lab(nc.sync.dma_start(
                                out=Gs[65:128, 0:ohc, :],
                                in_=Gt[64:127, 0:ohc, :],
                            ), f"p{p}c{c}sh2")
                            _lab(nc.sync.dma_start(
                                out=Gs[0:65:64, 0:ohc, :],
                                in_=Gt[0:65:64, 0:ohc, :],
                            ), f"p{p}c{c}sh3")
                    # ---- defer the D pool + cast + store by one chunk:
                    # by the time DVE reaches them, their DMA-completion
                    # semaphores (shift) are long satisfied, so the
                    # conservative vector-clock thresholds never stall ----
                    pend.append((p, c, G[0], G[1], Gs, ohr, ohc, s0))
                    if len(pend) > 2:
                        _flush_tail(pend.pop(0))
                    h0 += hc
            while pend:
                _flush_tail(pend.pop(0))
    nc.compile()
    return nc


def _get_nc():
    if "nc" not in _cache:
        _cache["nc"] = _build()
    return _cache["nc"]


def run(x: np.ndarray, **spmd_kwargs):
    """Run the SPMD kernel; returns the BassKernelResults (for tracing)."""
    from concourse.bass_utils import run_bass_kernel_spmd

    nc = _get_nc()
    xs = np.ascontiguousarray(x, dtype=np.float32).reshape(B * C, D, H, W)
    in_maps = [
        {"x_shard": np.ascontiguousarray(xs[SLICES_PER_CORE * i : SLICES_PER_CORE * (i + 1)])}
        for i in range(N_CORES)
    ]
    return run_bass_kernel_spmd(nc, in_maps, list(range(N_CORES)), **spmd_kwargs)


def kernel(x: np.ndarray) -> np.ndarray:
    res = run(x)
    out = np.stack([res.results[i]["y_shard"] for i in range(N_CORES)])
    return out.reshape(B, C, OD, OH, OW)
